# revision 1
# baseline (speedup 1.0000x reference)
"""Distributed causal attention block (QKV + RoPE + SDPA + Wo) on 8 TRN2 cores.

Sharding: tensor-parallel over heads (2 heads/core). Each core:
  phase 1: streams full x (bf16), PE-transposes tiles inline, QKV projection
           for its 2 heads + RoPE + transposes -> QT/KT/V resident in SBUF
  phase 2: causal attention per (batch, head), q-chunk-grouped PV with the
           softmax normalization folded into a P-prescale (on GpSimd)
  phase 3: AllGather attention outputs (d-sharded, in progressive t-pieces,
           tapering at the end) -> Wo e-slice -> output
Host concatenates the 8 e-slices.

The q/k columns of Wqkv (and the cos/sin tables) are permuted head-major
even/odd on the host so RoPE runs on contiguous blocks; attention scores are
invariant to a shared permutation of the head dim of Q and K.
"""
import numpy as np
import ml_dtypes
import bass_rust
import concourse.bass as bass
import concourse.mybir as mybir
from concourse.tile import TileContext, add_dep_helper
from concourse.masks import make_identity, make_causal_mask

B, L, D, H = 2, 2048, 2048, 16
HD = 128
N_CORES = 8
HPC = H // N_CORES          # heads per core = 2
ES = HPC * HD               # 256 = e-slice width per core
T = B * L                   # 4096 tokens total
TS = T // N_CORES           # 512 t per rank-block in phase 1
P = 128
SCALE = 1.0 / float(np.sqrt(HD))
NEG = -30000.0              # causal mask fill; exp(SCALE*(s+NEG)) underflows to 0
FP = mybir.dt.float32
BF = mybir.dt.bfloat16

N_TT = T // P               # 32 global t-tiles
N_LT = L // P               # 16 t-tiles per batch
N_DT = D // P               # 16 d-tiles

# attention-out AllGather pieces per batch, in units of 512-t q-chunks (4/batch)
AG_PIECES = {0: [(0, 2), (2, 4)], 1: [(0, 2), (2, 3), (3, 4)]}


def split_multi_waits(nc):
    """This walrus build allows 1 sync wait per instruction (2 for
    EventSemaphore). Tile attaches more on some instructions (tail drain,
    collective-adjacent DMAs); hoist the extras onto same-engine NoOps."""
    for f in nc.m.functions:
        for bb in f.blocks:
            new_insts = []
            changed = False
            for ins in bb.instructions:
                si = ins.sync_info
                cap = 2 if type(ins).__name__ == "InstEventSemaphore" else 1
                if si is not None and len(si.on_wait) > cap:
                    waits = list(si.on_wait)
                    for k, w in enumerate(waits[cap:]):
                        new_insts.append(mybir.InstNoOp(
                            name=f"{ins.name}-wsplit{k}", ins=[], outs=[],
                            engine=ins.engine,
                            sync_info=bass_rust.SyncInfo(on_wait=[w], on_update=[]),
                        ))
                    ins.sync_info = bass_rust.SyncInfo(
                        on_wait=waits[:cap], on_update=list(si.on_update))
                    changed = True
                new_insts.append(ins)
            if changed:
                bb.instructions.clear()
                for i2 in new_insts:
                    bb.add_instruction(i2)


def build(debug=False):
    nc = bass.Bass()
    x_c = nc.declare_dram_parameter("x_c", [T, D], BF, isOutput=False)
    wqkvT = nc.declare_dram_parameter("wqkvT", [D, 3 * ES], BF, isOutput=False)
    ce_p = nc.declare_dram_parameter("ce_p", [L, P], FP, isOutput=False)
    co_p = nc.declare_dram_parameter("co_p", [L, P], FP, isOutput=False)
    se_p = nc.declare_dram_parameter("se_p", [L, P], FP, isOutput=False)
    so_p = nc.declare_dram_parameter("so_p", [L, P], FP, isOutput=False)
    woT = nc.declare_dram_parameter("woT", [D, ES], BF, isOutput=False)
    out = nc.declare_dram_parameter("out", [ES, T], FP, isOutput=True)
    if debug:
        dbg_qt = nc.declare_dram_parameter("dbg_qt", [P, HPC * T], FP, isOutput=True)
        dbg_kt = nc.declare_dram_parameter("dbg_kt", [P, HPC * T], FP, isOutput=True)
        dbg_v = nc.declare_dram_parameter("dbg_v", [T, ES], FP, isOutput=True)
        dbg_o = nc.declare_dram_parameter("dbg_o", [ES, T], FP, isOutput=True)

    # out AllGather bounce/result per (batch, piece)
    o_bounce, ag_o = {}, {}
    for b, pieces in AG_PIECES.items():
        for (c0, c1) in pieces:
            w = (c1 - c0) * 512
            o_bounce[(b, c0)] = nc.dram_tensor(f"o_bounce{b}_{c0}", [ES, w], BF)
            ag_o[(b, c0)] = nc.dram_tensor(f"ag_o{b}_{c0}", [N_CORES * ES, w], BF,
                                           addr_space="Shared")
    rg = [list(range(N_CORES))]

    def r3(ap):  # [128, 256] -> [128, 2 heads, 2 (even/odd), 64]
        return ap.rearrange("p (h s x) -> p h s x", h=2, s=2)

    def r2(ap):  # [128, 128] -> [128, 2 heads, 64]
        return ap.rearrange("p (h x) -> p h x", h=2)

    with TileContext(nc, pool_alloc_mode="queue") as tc:
        with (
            tc.tile_pool(name="const", bufs=1) as const_pool,
            tc.tile_pool(name="resident", bufs=1) as res_pool,
            tc.tile_pool(name="wo", bufs=1) as wo_pool,
            tc.tile_pool(name="psA", bufs=2, space="PSUM") as psA,
            tc.tile_pool(name="psB", bufs=2, space="PSUM") as psB,
            tc.tile_pool(name="psC", bufs=2, space="PSUM") as psC,
            tc.tile_pool(name="psD", bufs=2, space="PSUM") as psD,
        ):
            ident = const_pool.tile([P, P], BF, name="ident")
            make_identity(nc, ident[:, :])
            cmask = const_pool.tile([P, P], FP, name="cmask")
            make_causal_mask(nc, cmask[:, :], mask_val=NEG)

            # resident through phases 1-2
            qt_sb = res_pool.tile([P, HPC * T], BF, name="qt_sb")   # [hd', h*T + t]
            kt_sb = res_pool.tile([P, HPC * T], BF, name="kt_sb")
            v_sb = res_pool.tile([P, N_TT * ES], BF, name="v_sb")   # [t%128, tt*ES+e]

            # ---------------- phase 1: x^T tiles + QKV + RoPE ----------------
            with (
                tc.tile_pool(name="wq", bufs=1) as wq_pool,
                tc.tile_pool(name="p1n", bufs=8) as p1n,
                tc.tile_pool(name="p1x", bufs=2) as p1x,
                tc.tile_pool(name="p1t", bufs=3) as p1t,
            ):
                wt_sb = wq_pool.tile([P, N_DT * 3 * ES], BF, name="wt_sb")
                trig_sb = {}
                for nm in ("ce", "co", "se", "so"):
                    trig_sb[nm] = wq_pool.tile([P, N_LT * P], FP, name=f"{nm}_sb")
                woT_sb = wo_pool.tile([P, N_DT * ES], BF, name="woT_sb")

                xins = {}

                def load_xins(rb):
                    tiles = []
                    for tl in range(TS // P):
                        xin = p1n.tile([P, D], BF, name="xin", tag="xin")
                        t0 = rb * TS + tl * P
                        nc.sync.dma_start(out=xin[:, :], in_=x_c[t0:t0 + P, :])
                        tiles.append(xin)
                    xins[rb] = tiles

                # priority: first two rank blocks of x, then weights, then trig
                load_xins(0)
                load_xins(1)
                for dt in range(N_DT):
                    nc.sync.dma_start(out=wt_sb[:, dt * 3 * ES:(dt + 1) * 3 * ES],
                                      in_=wqkvT[dt * P:(dt + 1) * P, :])
                for nm, prm in (("ce", ce_p), ("co", co_p),
                                ("se", se_p), ("so", so_p)):
                    for lt in range(N_LT):
                        nc.sync.dma_start(out=trig_sb[nm][:, lt * P:(lt + 1) * P],
                                          in_=prm[lt * P:(lt + 1) * P, :])

                for rb in range(N_CORES):
                    if rb + 2 < N_CORES:
                        pass  # xins loaded lazily below
                    # build x^T tiles for this rank block on the PE
                    xt_rb = p1x.tile([P, N_DT * TS], BF, name="xt_rb")
                    for dt in range(N_DT):
                        txp = psC.tile([P, TS], BF, name="txp", tag="C")
                        for tl in range(TS // P):
                            nc.tensor.transpose(
                                txp[:, tl * P:(tl + 1) * P],
                                xins[rb][tl][:, dt * P:(dt + 1) * P], ident[:, :])
                        nc.any.tensor_copy(xt_rb[:, dt * TS:(dt + 1) * TS],
                                           txp[:, :])
                    if rb + 2 < N_CORES:
                        load_xins(rb + 2)
                    if rb == N_CORES - 1:
                        for dt in range(N_DT):
                            nc.sync.dma_start(
                                out=woT_sb[:, dt * ES:(dt + 1) * ES],
                                in_=woT[dt * P:(dt + 1) * P, :])
                    for tl in range(TS // P):
                        tt = rb * (TS // P) + tl
                        lt = tt % N_LT
                        ps_qk = psA.tile([P, 2 * ES], FP, name="ps_qk", tag="A")
                        ps_v = psB.tile([P, ES], FP, name="ps_v", tag="B")
                        for dt in range(N_DT):
                            lhsT = xt_rb[:, dt * TS + tl * P: dt * TS + (tl + 1) * P]
                            nc.tensor.matmul(
                                ps_qk[:, :], lhsT,
                                wt_sb[:, dt * 3 * ES: dt * 3 * ES + 2 * ES],
                                start=(dt == 0), stop=(dt == N_DT - 1))
                            nc.tensor.matmul(
                                ps_v[:, :], lhsT,
                                wt_sb[:, dt * 3 * ES + 2 * ES:(dt + 1) * 3 * ES],
                                start=(dt == 0), stop=(dt == N_DT - 1))
                        nc.vector.tensor_copy(v_sb[:, tt * ES:(tt + 1) * ES],
                                              ps_v[:, :])
                        ce = r2(trig_sb["ce"][:, lt * P:(lt + 1) * P])
                        co = r2(trig_sb["co"][:, lt * P:(lt + 1) * P])
                        se = r2(trig_sb["se"][:, lt * P:(lt + 1) * P])
                        so = r2(trig_sb["so"][:, lt * P:(lt + 1) * P])
                        for part in range(2):  # 0=q, 1=k
                            src = r3(ps_qk[:, part * ES:(part + 1) * ES])
                            e_, o_ = src[:, :, 0, :], src[:, :, 1, :]
                            rot = p1t.tile([P, ES], BF, name="rot", tag="rot")
                            rdst = r3(rot[:, :])
                            re_, ro_ = rdst[:, :, 0, :], rdst[:, :, 1, :]
                            t1 = p1t.tile([P, P], FP, name="t1", tag="t1")
                            t2 = p1t.tile([P, P], FP, name="t2", tag="t2")
                            t13, t23 = r2(t1[:, :]), r2(t2[:, :])
                            nc.vector.tensor_tensor(t13, e_, ce,
                                                    op=mybir.AluOpType.mult)
                            nc.vector.tensor_tensor(t23, o_, se,
                                                    op=mybir.AluOpType.mult)
                            nc.vector.tensor_tensor(re_, t13, t23,
                                                    op=mybir.AluOpType.subtract)
                            nc.vector.tensor_tensor(t13, o_, co,
                                                    op=mybir.AluOpType.mult)
                            nc.vector.tensor_tensor(t23, e_, so,
                                                    op=mybir.AluOpType.mult)
                            nc.vector.tensor_tensor(ro_, t13, t23,
                                                    op=mybir.AluOpType.add)
                            dst = qt_sb if part == 0 else kt_sb
                            for h in range(HPC):
                                tps = psD.tile([P, P], BF, name="tps", tag="D")
                                nc.tensor.transpose(
                                    tps[:, :], rot[:, h * HD:(h + 1) * HD],
                                    ident[:, :])
                                nc.vector.tensor_copy(
                                    dst[:, h * T + tt * P: h * T + (tt + 1) * P],
                                    tps[:, :])

            if debug:
                with tc.tile_pool(name="dbgp", bufs=2) as dbgp:
                    for nm, src in (("dbg_qt", qt_sb), ("dbg_kt", kt_sb)):
                        dd = {"dbg_qt": dbg_qt, "dbg_kt": dbg_kt}[nm]
                        for i in range(HPC * T // 512):
                            s = dbgp.tile([P, 512], FP, name="dstage")
                            nc.vector.tensor_copy(s[:, :],
                                                  src[:, i * 512:(i + 1) * 512])
                            nc.sync.dma_start(out=dd[:, i * 512:(i + 1) * 512],
                                              in_=s[:, :])
                    for tt in range(N_TT):
                        s = dbgp.tile([P, ES], FP, name="dstage2")
                        nc.vector.tensor_copy(s[:, :], v_sb[:, tt * ES:(tt + 1) * ES])
                        nc.sync.dma_start(out=dbg_v[tt * P:(tt + 1) * P, :],
                                          in_=s[:, :])

            # ---------------- phases 2+3 (shared pools, interleaved) ----------
            with (
                tc.tile_pool(name="p2p", bufs=6) as p2p,
                tc.tile_pool(name="p2pt", bufs=3) as p2pt,
                tc.tile_pool(name="p2sm", bufs=4) as p2sm,
                tc.tile_pool(name="p2ob", bufs=2) as p2ob,
                tc.tile_pool(name="p3x", bufs=2) as p3x,
                tc.tile_pool(name="p3o", bufs=2) as p3o,
            ):
                ob_copies = {}

                def phase2(b):
                    ob_sb = p2ob.tile([P, HPC * L], BF, name="ob_sb", tag="ob")
                    for qc in range(4):
                        for h in range(HPC):
                            qoff = h * T + b * L
                            psbs = []
                            for qi in range(qc * 4, qc * 4 + 4):
                                kend = (qi + 1) * P
                                nch = (kend + 511) // 512
                                p_sb = p2p.tile([P, L], BF, name="p_sb", tag="p")
                                sums = p2sm.tile([P, 4], FP, name="sums", tag="sums")
                                for ci in range(nch):
                                    klo = ci * 512
                                    ksz = min(512, kend - klo)
                                    s_ps = psA.tile([P, 512], FP, name="s_ps", tag="A")
                                    nc.tensor.matmul(
                                        s_ps[:, :ksz],
                                        qt_sb[:, qoff + qi * P: qoff + (qi + 1) * P],
                                        kt_sb[:, qoff + klo: qoff + klo + ksz],
                                        start=True, stop=True)
                                    if klo + ksz == kend:  # diagonal 128-block
                                        dslice = s_ps[:, ksz - P:ksz]
                                        nc.vector.tensor_tensor(
                                            dslice, dslice, cmask[:, :],
                                            op=mybir.AluOpType.add)
                                    nc.scalar.activation(
                                        p_sb[:, klo:klo + ksz], s_ps[:, :ksz],
                                        mybir.ActivationFunctionType.Exp,
                                        scale=SCALE,
                                        accum_out=sums[:, ci:ci + 1])
                                tot = p2sm.tile([P, 1], FP, name="tot", tag="tot")
                                if nch > 1:
                                    nc.vector.tensor_reduce(
                                        tot[:, :], sums[:, :nch],
                                        axis=mybir.AxisListType.X,
                                        op=mybir.AluOpType.add)
                                else:
                                    nc.vector.tensor_copy(tot[:, :], sums[:, 0:1])
                                rec = p2sm.tile([P, 1], FP, name="rec", tag="rec")
                                nc.vector.reciprocal(rec[:, :], tot[:, :])
                                nc.vector.tensor_scalar_mul(
                                    p_sb[:, :kend], p_sb[:, :kend], rec[:, 0:1])
                                psbs.append(p_sb)
                            # PV for this q-chunk: o^T [hd, 512 q]
                            o_ps = psB.tile([P, 512], FP, name="o_ps", tag="B")
                            for kt in range(qc * 4 + 4):
                                off = max(0, kt * P - qc * 512)
                                pt_ps = psC.tile([P, 512], BF, name="pt_ps", tag="C")
                                for j in range(4):
                                    qi = qc * 4 + j
                                    if kt <= qi:
                                        nc.tensor.transpose(
                                            pt_ps[:, j * P:(j + 1) * P],
                                            psbs[j][:, kt * P:(kt + 1) * P],
                                            ident[:, :])
                                pt_sb = p2pt.tile([P, 512], BF, name="pt_sb")
                                nc.vector.tensor_copy(pt_sb[:, off:], pt_ps[:, off:])
                                nc.tensor.matmul(
                                    o_ps[:, off:],
                                    v_sb[:, (b * N_LT + kt) * ES + h * HD:
                                         (b * N_LT + kt) * ES + (h + 1) * HD],
                                    pt_sb[:, off:],
                                    start=(kt == 0), stop=(kt == qc * 4 + 3))
                            obcp = nc.vector.tensor_copy(
                                ob_sb[:, h * L + qc * 512:h * L + (qc + 1) * 512],
                                o_ps[:, :])
                            ob_copies[(b, qc)] = obcp
                        # piece boundary: bounce + AllGather
                        for (c0, c1) in AG_PIECES[b]:
                            if c1 == qc + 1:
                                for h in range(HPC):
                                    nc.sync.dma_start(
                                        out=o_bounce[(b, c0)][h * HD:(h + 1) * HD, :],
                                        in_=ob_sb[:, h * L + c0 * 512:
                                                  h * L + c1 * 512])
                                nc.gpsimd.collective_compute(
                                    "AllGather", mybir.AluOpType.bypass,
                                    ins=[o_bounce[(b, c0)][:]],
                                    outs=[ag_o[(b, c0)][:]],
                                    replica_groups=rg)

                def phase3(b, c0, c1, dep=None):
                    w = (c1 - c0) * 512
                    for tch in range(w // 512):
                        ot_ch = p3x.tile([P, N_DT * 512], BF, name="ot_ch")
                        for dt in range(N_DT):
                            d = nc.sync.dma_start(
                                out=ot_ch[:, dt * 512:(dt + 1) * 512],
                                in_=ag_o[(b, c0)][dt * P:(dt + 1) * P,
                                                  tch * 512:(tch + 1) * 512])
                            if dep is not None and tch == 0:
                                add_dep_helper(
                                    d.ins, dep.ins,
                                    reason="stagger ph3 behind real AG")
                        t0 = b * L + (c0 + tch) * 512
                        for et in range(2):
                            f_ps = psD.tile([P, 512], FP, name="f_ps", tag="D")
                            for dt in range(N_DT):
                                nc.tensor.matmul(
                                    f_ps[:, :],
                                    woT_sb[:, dt * ES + et * P:
                                           dt * ES + (et + 1) * P],
                                    ot_ch[:, dt * 512:(dt + 1) * 512],
                                    start=(dt == 0), stop=(dt == N_DT - 1))
                            f_sb = p3o.tile([P, 512], FP, name="f_sb")
                            nc.vector.tensor_copy(f_sb[:, :], f_ps[:, :])
                            nc.sync.dma_start(
                                out=out[et * P:(et + 1) * P, t0:t0 + 512],
                                in_=f_sb[:, :])

                phase2(0)
                phase2(1)
                phase3(0, 0, 2, dep=ob_copies[(0, 3)])
                phase3(0, 2, 4, dep=ob_copies[(1, 0)])
                phase3(1, 0, 2, dep=ob_copies[(1, 2)])
                phase3(1, 2, 3, dep=ob_copies[(1, 3)])
                phase3(1, 3, 4)

            if debug:
                with tc.tile_pool(name="dbgo", bufs=2) as dbgo:
                    for b, pieces in AG_PIECES.items():
                        for (c0, c1) in pieces:
                            w = (c1 - c0) * 512
                            for i in range(HPC):
                                s = dbgo.tile([P, 2048], FP, name="dob")
                                stg = dbgo.tile([P, 2048], BF, name="dob_b")
                                nc.sync.dma_start(
                                    out=stg[:, :w],
                                    in_=o_bounce[(b, c0)][i * HD:(i + 1) * HD, :])
                                nc.vector.tensor_copy(s[:, :w], stg[:, :w])
                                nc.sync.dma_start(
                                    out=dbg_o[i * HD:(i + 1) * HD,
                                              b * L + c0 * 512:
                                              b * L + c0 * 512 + w],
                                    in_=s[:, :w])

    split_multi_waits(nc)
    return nc


def make_in_maps(x, cos, sin, Wqkv, Wo):
    bf = ml_dtypes.bfloat16
    xf = np.ascontiguousarray(np.asarray(x).reshape(T, D)).astype(bf)
    # q/k column permutation: head-major, evens then odds
    perm = []
    for h in range(HPC):
        perm.extend(h * HD + 2 * np.arange(64))
        perm.extend(h * HD + 2 * np.arange(64) + 1)
    perm = np.asarray(perm)
    epick = np.concatenate([h * HD + 2 * np.arange(64) for h in range(HPC)])
    in_maps = []
    for c in range(N_CORES):
        cols = slice(c * ES, (c + 1) * ES)
        wq = Wqkv[c * ES:(c + 1) * ES, :][perm]
        wk = Wqkv[D + c * ES: D + (c + 1) * ES, :][perm]
        wv = Wqkv[2 * D + c * ES: 2 * D + (c + 1) * ES, :]
        w_c = np.concatenate([wq, wk, wv], axis=0)
        cos_c = np.asarray(cos)[:, cols]
        sin_c = np.asarray(sin)[:, cols]
        in_maps.append({
            "x_c": xf,
            "wqkvT": np.ascontiguousarray(w_c.T.astype(bf)),
            "ce_p": np.ascontiguousarray(cos_c[:, epick]).astype(np.float32),
            "co_p": np.ascontiguousarray(cos_c[:, epick + 1]).astype(np.float32),
            "se_p": np.ascontiguousarray(sin_c[:, epick]).astype(np.float32),
            "so_p": np.ascontiguousarray(sin_c[:, epick + 1]).astype(np.float32),
            "woT": np.ascontiguousarray(Wo[cols, :].T.astype(bf)),
        })
    return in_maps


_cache = {}


def kernel(x, cos, sin, Wqkv, Wo):
    from concourse.bass_utils import run_bass_kernel_spmd
    x = np.asarray(x, dtype=np.float32)
    cos = np.asarray(cos, dtype=np.float32)
    sin = np.asarray(sin, dtype=np.float32)
    Wqkv = np.asarray(Wqkv, dtype=np.float32)
    Wo = np.asarray(Wo, dtype=np.float32)
    if "nc" not in _cache:
        _cache["nc"] = build()
    nc = _cache["nc"]
    in_maps = make_in_maps(x, cos, sin, Wqkv, Wo)
    res = run_bass_kernel_spmd(nc, in_maps, core_ids=list(range(N_CORES)))
    pieces = [res.results[c]["out"].T for c in range(N_CORES)]
    return np.concatenate(pieces, axis=1).reshape(B, L, D)



# revision 2
# speedup vs baseline: 1.2276x; 1.2276x over previous
"""Distributed causal attention block (QKV + RoPE + SDPA + Wo) on 8 TRN2 cores.

Tensor-parallel over heads (2 heads/core). Per core, all PE transposes are
eliminated by layout choices:
  - host supplies x^T in d-tile-blocked form; QKV for q/k is computed
    weight-stationary directly into [e, t] layout (qT/kT resident in SBUF),
    RoPE runs in [e, t] with cross-partition DVE ops; V is computed
    x-stationary directly into [t, e] layout.
  - attention scores are computed directly transposed: sT[k, q] via
    lhsT=kT-tile, rhs=qT; exp on ScalarE writes pT tiles; PV and the softmax
    denominator (ones-matmul with M=128 -> sums replicated on every
    partition) consume pT straight from SBUF; normalization is fused into
    the single O-drain DVE op (o * recip, psum -> sbuf bf16).
  - attention output O^T is AllGathered in pieces; Wo e-slice per core.
Emission interleaves attention chunks with QKV chunks and Wo pieces so the
PE stays dense while ScalarE works through the softmax exps.
"""
import numpy as np
import ml_dtypes
import bass_rust
import concourse.bass as bass
import concourse.mybir as mybir
from concourse.tile import TileContext, add_dep_helper

B, L, D, H = 2, 2048, 2048, 16
HD = 128
N_CORES = 8
HPC = H // N_CORES          # heads per core = 2
ES = HPC * HD               # 256 = e-slice width per core
T = B * L                   # 4096 tokens
P = 128
CH = 512                    # t-chunk width in phase 1 / q-chunk in attention
N_CH = T // CH              # 8 chunks
N_DT = D // P               # 16 d-tiles
N_TT = T // P               # 32 t-tiles
N_LT = L // P               # 16 k-tiles per batch
SCALE = 1.0 / float(np.sqrt(HD))
NEG = -30000.0
FP = mybir.dt.float32
BF = mybir.dt.bfloat16

# AllGather pieces per batch, in units of 512-q chunks
AG_PIECES = {0: [(0, 2), (2, 4)], 1: [(0, 2), (2, 3), (3, 4)]}


def split_multi_waits(nc):
    """This walrus build allows 1 sync wait per instruction (2 for
    EventSemaphore). Tile attaches more on some instructions; hoist the
    extras onto same-engine NoOps."""
    for f in nc.m.functions:
        for bb in f.blocks:
            new_insts = []
            changed = False
            for ins in bb.instructions:
                si = ins.sync_info
                cap = 2 if type(ins).__name__ == "InstEventSemaphore" else 1
                if si is not None and len(si.on_wait) > cap:
                    waits = list(si.on_wait)
                    for kk, w in enumerate(waits[cap:]):
                        new_insts.append(mybir.InstNoOp(
                            name=f"{ins.name}-wsplit{kk}", ins=[], outs=[],
                            engine=ins.engine,
                            sync_info=bass_rust.SyncInfo(on_wait=[w], on_update=[]),
                        ))
                    ins.sync_info = bass_rust.SyncInfo(
                        on_wait=waits[:cap], on_update=list(si.on_update))
                    changed = True
                new_insts.append(ins)
            if changed:
                bb.instructions.clear()
                for i2 in new_insts:
                    bb.add_instruction(i2)


def build(hw=True):
    nc = bass.Bass()
    # host-blocked x^T: x_d[ki, tc*16*CH + j*CH + t'] = x[tc*CH+t', j*128+ki]
    x_d = nc.declare_dram_parameter("x_d", [P, N_CH * N_DT * CH], BF, isOutput=False)
    # w_d[ki, j*768 + e] = Wcat^T[j*128+ki, e]; Wcat rows = [qh0 qh1 kh0 kh1 v]
    w_d = nc.declare_dram_parameter("w_d", [P, N_DT * 3 * ES], BF, isOutput=False)
    trig_d = {}
    for h in range(HPC):
        trig_d[("c", h)] = nc.declare_dram_parameter(f"ce{h}", [P, L], BF, isOutput=False)
        trig_d[("s", h)] = nc.declare_dram_parameter(f"ss{h}", [P, L], BF, isOutput=False)
    woT_d = nc.declare_dram_parameter("woT", [P, N_DT * ES], BF, isOutput=False)
    out_d = nc.declare_dram_parameter("out", [ES, T], FP, isOutput=True)

    o_bounce, ag_o = {}, {}
    for b, pieces in AG_PIECES.items():
        for (c0, c1) in pieces:
            w = (c1 - c0) * CH
            o_bounce[(b, c0)] = nc.dram_tensor(f"o_bounce{b}_{c0}", [ES, w], BF)
            ag_o[(b, c0)] = nc.dram_tensor(f"ag_o{b}_{c0}", [N_CORES * ES, w], BF,
                                           addr_space="Shared")
    rg = [list(range(N_CORES))]

    with TileContext(nc, pool_alloc_mode="queue") as tc:
        with (
            tc.tile_pool(name="const", bufs=1) as const_pool,
            tc.tile_pool(name="res", bufs=1) as res_pool,
            tc.tile_pool(name="wo", bufs=1) as wo_pool,
            tc.tile_pool(name="pT", bufs=2) as pT_pool,
            tc.tile_pool(name="ob", bufs=2) as ob_pool,
            tc.tile_pool(name="rec", bufs=2) as rec_pool,
            tc.tile_pool(name="psA", bufs=2, space="PSUM") as psA,
            tc.tile_pool(name="psB", bufs=3, space="PSUM") as psB,
            tc.tile_pool(name="psO", bufs=2, space="PSUM") as psO,
            tc.tile_pool(name="psS", bufs=1, space="PSUM") as psS,
        ):
            cmaskT = const_pool.tile([P, P], FP, name="cmaskT")
            nc.gpsimd.memset(cmaskT[:, :], 0.0)
            # NEG where k (partition) > q (free)
            nc.gpsimd.affine_select(
                out=cmaskT[:, :], in_=cmaskT[:, :],
                compare_op=mybir.AluOpType.is_ge, fill=NEG,
                base=0, pattern=[[1, P]], channel_multiplier=-1)
            ones_sb = const_pool.tile([P, P], BF, name="ones_sb")
            nc.vector.memset(ones_sb[:, :], 1.0)

            qt_sb = res_pool.tile([P, HPC * T], BF, name="qt_sb")  # [hd, h*T+t]
            kt_sb = res_pool.tile([P, HPC * T], BF, name="kt_sb")
            v_sb = res_pool.tile([P, N_TT * ES], BF, name="v_sb")  # [t%128, tt*ES+e]
            v3 = v_sb.rearrange("p (tt e) -> p tt e", e=ES)
            woT_sb = wo_pool.tile([P, N_DT * ES], BF, name="woT_sb")
            wo3 = woT_sb.rearrange("p (j e) -> p j e", j=N_DT)

            ob_sb = {}
            ag_deps = {}

            def emit_attn_qk(b, h, qc, pT):
                """QK matmuls + exp into pT for (b, h, q-chunk qc)."""
                qoff = h * T + b * L + qc * CH
                koff = h * T + b * L
                nkt = 4 * qc + 4
                for kt in range(nkt):
                    off = max(0, kt * P - qc * CH)
                    sps = psA.tile([P, CH], FP, name="sps", tag="A")
                    nc.tensor.matmul(
                        sps[:, off:], kt_sb[:, koff + kt * P: koff + (kt + 1) * P],
                        qt_sb[:, qoff + off: qoff + CH], start=True, stop=True)
                    if kt >= 4 * qc:  # diagonal 128-block
                        nc.vector.tensor_tensor(
                            sps[:, off:off + P], sps[:, off:off + P], cmaskT[:, :],
                            op=mybir.AluOpType.add)
                    nc.scalar.activation(
                        pT[:, kt, off:], sps[:, off:],
                        mybir.ActivationFunctionType.Exp, scale=SCALE)
                    if off:
                        nc.vector.memset(pT[:, kt, :off], 0.0)

            def emit_attn_pv(b, h, qc, pT):
                """PV + replicated-sums + fused normalize into ob_sb."""
                nkt = 4 * qc + 4
                ops = psO.tile([P, CH], FP, name="ops", tag="O")
                sps2 = psS.tile([P, CH], FP, name="sps2", tag="S")
                for kt in range(nkt):
                    nc.tensor.matmul(
                        ops[:, :], v3[:, b * N_LT + kt, h * HD:(h + 1) * HD],
                        pT[:, kt, :], start=(kt == 0), stop=(kt == nkt - 1))
                    nc.tensor.matmul(
                        sps2[:, :], ones_sb[:, :], pT[:, kt, :],
                        start=(kt == 0), stop=(kt == nkt - 1))
                rec = rec_pool.tile([P, CH], FP, name="rec", tag="rec")
                nc.vector.reciprocal(rec[:, :], sps2[:, :])
                obcp = nc.vector.tensor_tensor(
                    ob_sb[b][:, h * L + qc * CH: h * L + (qc + 1) * CH],
                    ops[:, :], rec[:, :], op=mybir.AluOpType.mult)
                ag_deps[(b, h, qc)] = obcp

            def post_ag(b, c0, c1):
                for h in range(HPC):
                    nc.sync.dma_start(
                        out=o_bounce[(b, c0)][h * HD:(h + 1) * HD, :],
                        in_=ob_sb[b][:, h * L + c0 * CH: h * L + c1 * CH])
                nc.gpsimd.collective_compute(
                    "AllGather", mybir.AluOpType.bypass,
                    ins=[o_bounce[(b, c0)][:]],
                    outs=[ag_o[(b, c0)][:]],
                    replica_groups=rg)

            # ------- phase-1 unit generators (QKV for one t-chunk) -------
            with (
                tc.tile_pool(name="w1", bufs=1) as w1_pool,
                tc.tile_pool(name="xin", bufs=2) as xin_pool,
                tc.tile_pool(name="rtmp", bufs=2) as rtmp_pool,
            ):
                w_sb = w1_pool.tile([P, N_DT * 3 * ES], BF, name="w_sb")
                w3 = w_sb.rearrange("p (j e) -> p j e", j=N_DT)
                trig_sb = {}
                for key, prm in trig_d.items():
                    t_ = w1_pool.tile([P, L], BF, name=f"trig_{key[0]}{key[1]}")
                    trig_sb[key] = t_

                def load_weights():
                    nc.sync.dma_start(out=w_sb[:, :], in_=w_d[:, :])
                    for key, prm in trig_d.items():
                        nc.sync.dma_start(out=trig_sb[key][:, :], in_=prm[:, :])
                    nc.sync.dma_start(out=woT_sb[:, :], in_=woT_d[:, :])

                xins = {}

                def load_x(tc_):
                    xin = xin_pool.tile([P, N_DT * CH], BF, name="xin", tag="xin")
                    nc.sync.dma_start(
                        out=xin[:, :],
                        in_=x_d[:, tc_ * N_DT * CH:(tc_ + 1) * N_DT * CH])
                    xins[tc_] = xin.rearrange("p (j t) -> p j t", j=N_DT)

                def ph1_qk_group(tc_, g):
                    """g in 0..3: qh0, qh1, kh0, kh1 -> [e,t] psum + RoPE."""
                    x3 = xins[tc_]
                    h = g % 2
                    is_k = g // 2
                    ps = psB.tile([P, CH], FP, name="ps_qk", tag="B")
                    for dt in range(N_DT):
                        nc.tensor.matmul(
                            ps[:, :], w3[:, dt, g * P:(g + 1) * P], x3[:, dt, :],
                            start=(dt == 0), stop=(dt == N_DT - 1))
                    l0 = (tc_ % (L // CH)) * CH
                    ce = trig_sb[("c", h)][:, l0:l0 + CH]
                    ss = trig_sb[("s", h)][:, l0:l0 + CH]
                    t1 = rtmp_pool.tile([P, CH], FP, name="t1", tag="t1")
                    t2 = rtmp_pool.tile([P, CH], FP, name="t2", tag="t2")
                    nc.vector.tensor_tensor(t1[:, :], ps[:, :], ce,
                                            op=mybir.AluOpType.mult)
                    nc.vector.tensor_tensor(t2[0:64, :], ps[64:128, :], ss[0:64, :],
                                            op=mybir.AluOpType.mult)
                    nc.vector.tensor_tensor(t2[64:128, :], ps[0:64, :], ss[64:128, :],
                                            op=mybir.AluOpType.mult)
                    dst = kt_sb if is_k else qt_sb
                    nc.vector.tensor_tensor(
                        dst[:, h * T + tc_ * CH: h * T + (tc_ + 1) * CH],
                        t1[:, :], t2[:, :], op=mybir.AluOpType.add)

                def ph1_v_tile(tc_, tl):
                    """V for t-tile tl of chunk tc_, x-stationary -> [t, e]."""
                    x3 = xins[tc_]
                    ps = psB.tile([P, ES], FP, name="ps_v", tag="B",
                                  padded_shape=[P, CH])
                    for dt in range(N_DT):
                        nc.tensor.matmul(
                            ps[:, :], x3[:, dt, tl * P:(tl + 1) * P],
                            w3[:, dt, 2 * ES:3 * ES],
                            start=(dt == 0), stop=(dt == N_DT - 1))
                    tt = tc_ * 4 + tl
                    nc.vector.tensor_copy(v_sb[:, tt * ES:(tt + 1) * ES], ps[:, :])

                def ph1_chunk(tc_):
                    for g in range(4):
                        ph1_qk_group(tc_, g)
                    for tl in range(4):
                        ph1_v_tile(tc_, tl)

                # ---------------- emission schedule ----------------
                load_x(0)
                load_x(1)
                load_weights()
                ob_sb[0] = ob_pool.tile([P, HPC * L], BF, name="ob_sb0", tag="ob")
                ob_sb[1] = ob_pool.tile([P, HPC * L], BF, name="ob_sb1", tag="ob")

                # filler queue of phase-1 units for chunks 1..7
                fillers = []
                for tc_ in range(1, N_CH):
                    for g in range(4):
                        fillers.append((ph1_qk_group, tc_, g))
                    for tl in range(4):
                        fillers.append((ph1_v_tile, tc_, tl))

                def fill(n):
                    for _ in range(n):
                        if fillers:
                            fn, a1, a2 = fillers.pop(0)
                            fn(a1, a2)

                def drain_ph1_through(tc_needed):
                    """emit remaining units of chunks <= tc_needed."""
                    while fillers and fillers[0][1] <= tc_needed:
                        fn, a1, a2 = fillers.pop(0)
                        fn(a1, a2)

                ph1_chunk(0)
                load_x(2)

                pT_tiles = {}
                for b in range(B):
                    for qc in range(4):
                        # ensure needed chunks are emitted
                        drain_ph1_through(b * 4 + qc)
                        if b * 4 + qc + 3 < N_CH:
                            load_x(b * 4 + qc + 3)
                        for h in range(HPC):
                            pT = pT_pool.tile([P, N_LT, CH], BF, name="pT", tag="pT")
                            emit_attn_qk(b, h, qc, pT)
                            fill(3)
                            emit_attn_pv(b, h, qc, pT)
                        for (c0, c1) in AG_PIECES[b]:
                            if c1 == qc + 1:
                                post_ag(b, c0, c1)

                drain_ph1_through(N_CH - 1)

            # ---------------- phase 3: Wo pieces ----------------
            with (
                tc.tile_pool(name="ot", bufs=2) as ot_pool,
                tc.tile_pool(name="fo", bufs=2) as fo_pool,
            ):
                def ph3_piece(b, c0, c1, dep=None):
                    for tch in range(c1 - c0):
                        ot = ot_pool.tile([P, N_DT, CH], BF, name="ot", tag="ot")
                        for dt in range(N_DT):
                            d = nc.sync.dma_start(
                                out=ot[:, dt, :],
                                in_=ag_o[(b, c0)][dt * P:(dt + 1) * P,
                                                  tch * CH:(tch + 1) * CH])
                            if dep is not None and tch == 0:
                                add_dep_helper(d.ins, dep.ins,
                                               reason="stagger ph3 behind AG")
                        t0 = b * L + (c0 + tch) * CH
                        for et in range(HPC):
                            f_ps = psB.tile([P, CH], FP, name="f_ps", tag="B")
                            for dt in range(N_DT):
                                nc.tensor.matmul(
                                    f_ps[:, :], wo3[:, dt, et * P:(et + 1) * P],
                                    ot[:, dt, :],
                                    start=(dt == 0), stop=(dt == N_DT - 1))
                            f_sb = fo_pool.tile([P, CH], FP, name="f_sb", tag="f")
                            nc.vector.tensor_copy(f_sb[:, :], f_ps[:, :])
                            nc.sync.dma_start(
                                out=out_d[et * P:(et + 1) * P, t0:t0 + CH],
                                in_=f_sb[:, :])

                ph3_piece(0, 0, 2, dep=ag_deps[(1, 1, 0)])
                ph3_piece(0, 2, 4, dep=ag_deps[(1, 1, 1)])
                ph3_piece(1, 0, 2, dep=ag_deps[(1, 1, 3)])
                ph3_piece(1, 2, 3)
                ph3_piece(1, 3, 4)

    if hw:
        split_multi_waits(nc)
    return nc


def make_in_maps(x, cos, sin, Wqkv, Wo):
    bf16 = ml_dtypes.bfloat16
    xf = np.ascontiguousarray(np.asarray(x).reshape(T, D)).astype(np.float32)
    # x blocked: [ki, tc, j, t'] = x[tc*CH+t', j*128+ki]
    xT = xf.T  # [D, T]
    xb = np.ascontiguousarray(
        xT.reshape(N_DT, P, N_CH, CH).transpose(1, 2, 0, 3).reshape(P, -1)
    ).astype(bf16)

    # head-major even/odd permutation within each head's 128 dims
    perm = []
    for h in range(HPC):
        perm.extend(h * HD + 2 * np.arange(64))
        perm.extend(h * HD + 2 * np.arange(64) + 1)
    perm = np.asarray(perm)

    cos = np.asarray(cos)
    sin = np.asarray(sin)
    in_maps = []
    for c in range(N_CORES):
        rows = slice(c * ES, (c + 1) * ES)
        wq = Wqkv[rows][perm]          # [256, D] head-major evens/odds
        wk = Wqkv[D + c * ES: D + (c + 1) * ES][perm]
        wv = Wqkv[2 * D + c * ES: 2 * D + (c + 1) * ES]
        wcat = np.concatenate([wq, wk, wv], axis=0)    # [768, D]
        # w blocked: [ki, j, e] = wcat[e, j*128+ki]
        wb = np.ascontiguousarray(
            wcat.T.reshape(N_DT, P, 3 * ES).transpose(1, 0, 2).reshape(P, -1)
        ).astype(bf16)
        im = {"x_d": xb, "w_d": wb}
        for h in range(HPC):
            gh = c * HPC + h
            ce = cos[:, gh * HD + 2 * np.arange(64)].T          # [64, L]
            ss = sin[:, gh * HD + 2 * np.arange(64)].T
            im[f"ce{h}"] = np.ascontiguousarray(
                np.concatenate([ce, ce], axis=0)).astype(bf16)
            im[f"ss{h}"] = np.ascontiguousarray(
                np.concatenate([-ss, ss], axis=0)).astype(bf16)
        # woT blocked: [ki, j, e] = Wo[c*ES+e, j*128+ki]
        wo = Wo[rows]                                   # [256, D]
        wob = np.ascontiguousarray(
            wo.T.reshape(N_DT, P, ES).transpose(1, 0, 2).reshape(P, -1)
        ).astype(bf16)
        im["woT"] = wob
        in_maps.append(im)
    return in_maps


_cache = {}


def kernel(x, cos, sin, Wqkv, Wo):
    from concourse.bass_utils import run_bass_kernel_spmd
    x = np.asarray(x, dtype=np.float32)
    cos = np.asarray(cos, dtype=np.float32)
    sin = np.asarray(sin, dtype=np.float32)
    Wqkv = np.asarray(Wqkv, dtype=np.float32)
    Wo = np.asarray(Wo, dtype=np.float32)
    if "nc" not in _cache:
        _cache["nc"] = build()
    nc = _cache["nc"]
    in_maps = make_in_maps(x, cos, sin, Wqkv, Wo)
    res = run_bass_kernel_spmd(nc, in_maps, core_ids=list(range(N_CORES)))
    pieces = [res.results[c]["out"].T for c in range(N_CORES)]
    return np.concatenate(pieces, axis=1).reshape(B, L, D)


# revision 6
# speedup vs baseline: 1.2539x; 1.0214x over previous
"""Distributed causal attention block (QKV + RoPE + SDPA + Wo) on 8 TRN2 cores.

Tensor-parallel over heads (2 heads/core). Per core, all PE transposes are
eliminated by layout choices:
  - host supplies x^T in d-tile-blocked form; QKV for q/k is computed
    weight-stationary directly into [e, t] layout (qT/kT resident in SBUF),
    RoPE runs in [e, t] with cross-partition DVE ops; V is computed
    x-stationary directly into [t, e] layout.
  - attention scores are computed directly transposed: sT[k, q] via
    lhsT=kT-tile, rhs=qT; exp on ScalarE writes pT tiles; the causal mask is
    applied AFTER exp by a GpSimd affine_select zeroing the upper triangle of
    the diagonal pT block (keeps the QK->exp chain free of DVE); PV and the
    softmax denominator (ones-matmul with M=128 -> sums replicated on every
    partition) consume pT straight from SBUF; normalization is fused into the
    single O-drain DVE op (o * recip, psum -> sbuf bf16).
  - attention output O^T is AllGathered in pieces; Wo e-slice per core.
Emission interleaves at matmul granularity: QKV matmuls are zipped between
score matmuls (whose issue rate is bounded by ScalarE's exp), and Wo pieces
of batch 0 fill batch-1 attention windows.
"""
import numpy as np
import ml_dtypes
import bass_rust
import concourse.bass as bass
import concourse.mybir as mybir
from concourse.tile import TileContext, add_dep_helper

B, L, D, H = 2, 2048, 2048, 16
HD = 128
N_CORES = 8
HPC = H // N_CORES          # heads per core = 2
ES = HPC * HD               # 256 = e-slice width per core
T = B * L                   # 4096 tokens
P = 128
CH = 512                    # t-chunk width in phase 1 / q-chunk in attention
N_CH = T // CH              # 8 chunks
N_DT = D // P               # 16 d-tiles
N_TT = T // P               # 32 t-tiles
N_LT = L // P               # 16 k-tiles per batch
SCALE = 1.0 / float(np.sqrt(HD))
FP = mybir.dt.float32
BF = mybir.dt.bfloat16

# AllGather pieces per batch, in units of 512-q chunks
AG_PIECES = {0: [(0, 2), (2, 4)], 1: [(0, 2), (2, 3), (3, 4)]}


def split_multi_waits(nc):
    """This walrus build allows 1 sync wait per instruction (2 for
    EventSemaphore). Tile attaches more on some instructions; hoist the
    extras onto same-engine NoOps."""
    for f in nc.m.functions:
        for bb in f.blocks:
            new_insts = []
            changed = False
            for ins in bb.instructions:
                si = ins.sync_info
                cap = 2 if type(ins).__name__ == "InstEventSemaphore" else 1
                if si is not None and len(si.on_wait) > cap:
                    waits = list(si.on_wait)
                    for kk, w in enumerate(waits[cap:]):
                        new_insts.append(mybir.InstNoOp(
                            name=f"{ins.name}-wsplit{kk}", ins=[], outs=[],
                            engine=ins.engine,
                            sync_info=bass_rust.SyncInfo(on_wait=[w], on_update=[]),
                        ))
                    ins.sync_info = bass_rust.SyncInfo(
                        on_wait=waits[:cap], on_update=list(si.on_update))
                    changed = True
                new_insts.append(ins)
            if changed:
                bb.instructions.clear()
                for i2 in new_insts:
                    bb.add_instruction(i2)


def build(hw=True):
    nc = bass.Bass()
    x_d = nc.declare_dram_parameter("x_d", [P, N_CH * N_DT * CH], BF, isOutput=False)
    w_d = nc.declare_dram_parameter("w_d", [P, N_DT * 3 * ES], BF, isOutput=False)
    trig_d = {}
    for h in range(HPC):
        trig_d[("c", h)] = nc.declare_dram_parameter(f"ce{h}", [P, L], BF, isOutput=False)
        trig_d[("s", h)] = nc.declare_dram_parameter(f"ss{h}", [P, L], BF, isOutput=False)
    woT_d = nc.declare_dram_parameter("woT", [P, N_DT * ES], BF, isOutput=False)
    out_d = nc.declare_dram_parameter("out", [ES, T], FP, isOutput=True)

    o_bounce, ag_o = {}, {}
    for b, pieces in AG_PIECES.items():
        for (c0, c1) in pieces:
            w = (c1 - c0) * CH
            o_bounce[(b, c0)] = nc.dram_tensor(f"o_bounce{b}_{c0}", [ES, w], BF)
            ag_o[(b, c0)] = nc.dram_tensor(f"ag_o{b}_{c0}", [N_CORES * ES, w], BF,
                                           addr_space="Shared")
    rg = [list(range(N_CORES))]

    with TileContext(nc, pool_alloc_mode="queue") as tc:
        with (
            tc.tile_pool(name="const", bufs=1) as const_pool,
            tc.tile_pool(name="res", bufs=1) as res_pool,
            tc.tile_pool(name="wo", bufs=1) as wo_pool,
            tc.tile_pool(name="pT", bufs=2) as pT_pool,
            tc.tile_pool(name="ob", bufs=2) as ob_pool,
            tc.tile_pool(name="rec", bufs=2) as rec_pool,
            tc.tile_pool(name="psA", bufs=2, space="PSUM") as psA,
            tc.tile_pool(name="psB", bufs=3, space="PSUM") as psB,
            tc.tile_pool(name="psO", bufs=2, space="PSUM") as psO,
            tc.tile_pool(name="psS", bufs=1, space="PSUM") as psS,
        ):
            ones_sb = const_pool.tile([P, P], BF, name="ones_sb")
            nc.vector.memset(ones_sb[:, :], 1.0)

            qt_sb = res_pool.tile([P, HPC * T], BF, name="qt_sb")  # [hd, h*T+t]
            kt_sb = res_pool.tile([P, HPC * T], BF, name="kt_sb")
            v_sb = res_pool.tile([P, N_TT * ES], BF, name="v_sb")  # [t%128, tt*ES+e]
            v3 = v_sb.rearrange("p (tt e) -> p tt e", e=ES)
            woT_sb = wo_pool.tile([P, N_DT * ES], BF, name="woT_sb")
            wo3 = woT_sb.rearrange("p (j e) -> p j e", j=N_DT)

            # -------- phase-1 pools (released mid-stream) --------
            w1_pool = tc.alloc_tile_pool(name="w1", bufs=1)
            xin_pool = tc.alloc_tile_pool(name="xin", bufs=2)
            rtmp_pool = tc.alloc_tile_pool(name="rtmp", bufs=2)

            w_sb = w1_pool.tile([P, N_DT * 3 * ES], BF, name="w_sb")
            w3 = w_sb.rearrange("p (j e) -> p j e", j=N_DT)
            w_d3 = w_d[:, :].rearrange("p (j e) -> p j e", j=N_DT)
            trig_sb = {}
            for key in trig_d:
                trig_sb[key] = w1_pool.tile([P, L], BF, name=f"trig_{key[0]}{key[1]}")

            xins = {}
            loaded = set()

            def load_x(tc_, split=False):
                if tc_ in loaded or tc_ >= N_CH:
                    return
                loaded.add(tc_)
                xin = xin_pool.tile([P, N_DT * CH], BF, name="xin", tag="xin")
                base = tc_ * N_DT * CH
                if split:  # halve first-chunk latency across both HWDGE queues
                    half = N_DT * CH // 2
                    nc.sync.dma_start(out=xin[:, :half],
                                      in_=x_d[:, base:base + half])
                    nc.scalar.dma_start(out=xin[:, half:],
                                        in_=x_d[:, base + half:base + 2 * half])
                else:
                    nc.sync.dma_start(
                        out=xin[:, :],
                        in_=x_d[:, base:(tc_ + 1) * N_DT * CH])
                xins[tc_] = xin.rearrange("p (j t) -> p j t", j=N_DT)

            def ph1_qk_unit(tc_, g):
                """generator: 16 matmuls then RoPE drain. g: qh0,qh1,kh0,kh1"""
                x3 = xins[tc_]
                h = g % 2
                is_k = g // 2
                ps = psB.tile([P, CH], FP, name="ps_qk", tag="B")
                for dt in range(N_DT):
                    nc.tensor.matmul(
                        ps[:, :], w3[:, dt, g * P:(g + 1) * P], x3[:, dt, :],
                        start=(dt == 0), stop=(dt == N_DT - 1))
                    yield
                l0 = (tc_ % (L // CH)) * CH
                ce = trig_sb[("c", h)][:, l0:l0 + CH]
                ss = trig_sb[("s", h)][:, l0:l0 + CH]
                t1 = rtmp_pool.tile([P, CH], FP, name="t1", tag="t1")
                t2 = rtmp_pool.tile([P, CH], FP, name="t2", tag="t2")
                nc.vector.tensor_tensor(t1[:, :], ps[:, :], ce,
                                        op=mybir.AluOpType.mult)
                nc.vector.tensor_tensor(t2[0:64, :], ps[64:128, :], ss[0:64, :],
                                        op=mybir.AluOpType.mult)
                nc.vector.tensor_tensor(t2[64:128, :], ps[0:64, :], ss[64:128, :],
                                        op=mybir.AluOpType.mult)
                dst = kt_sb if is_k else qt_sb
                nc.vector.tensor_tensor(
                    dst[:, h * T + tc_ * CH: h * T + (tc_ + 1) * CH],
                    t1[:, :], t2[:, :], op=mybir.AluOpType.add)

            def ph1_v_unit(tc_, tl):
                """generator: V for t-tile tl of chunk tc_ -> [t, e]"""
                x3 = xins[tc_]
                ps = psB.tile([P, ES], FP, name="ps_v", tag="B",
                              padded_shape=[P, CH])
                for dt in range(N_DT):
                    nc.tensor.matmul(
                        ps[:, :], x3[:, dt, tl * P:(tl + 1) * P],
                        w3[:, dt, 2 * ES:3 * ES],
                        start=(dt == 0), stop=(dt == N_DT - 1))
                    yield
                tt = tc_ * 4 + tl
                nc.vector.tensor_copy(v_sb[:, tt * ES:(tt + 1) * ES], ps[:, :])

            # filler stream over phase-1 units, chunks 1..7
            unit_specs = []
            for tc_ in range(1, N_CH):
                for g in range(4):
                    unit_specs.append((ph1_qk_unit, tc_, g))
                for tl in range(4):
                    unit_specs.append((ph1_v_unit, tc_, tl))
            state = {"gen": None, "chunk": 0, "idx": 0, "released": False}

            def _next_unit():
                if state["idx"] >= len(unit_specs):
                    state["gen"] = None
                    return False
                fn, a1, a2 = unit_specs[state["idx"]]
                state["idx"] += 1
                load_x(a1)
                load_x(a1 + 1)
                state["gen"] = fn(a1, a2)
                state["chunk"] = a1
                return True

            def fill_mms(n):
                """emit up to n matmuls from the phase-1 unit stream."""
                while n > 0:
                    if state["gen"] is None and not _next_unit():
                        return
                    try:
                        next(state["gen"])
                        n -= 1
                    except StopIteration:
                        state["gen"] = None

            def drain_ph1_through(tc_needed):
                while True:
                    if state["gen"] is not None and state["chunk"] <= tc_needed:
                        for _ in state["gen"]:
                            pass
                        state["gen"] = None
                        continue
                    if state["gen"] is None:
                        if (state["idx"] < len(unit_specs)
                                and unit_specs[state["idx"]][1] <= tc_needed):
                            _next_unit()
                            continue
                    break

            def ph1_done():
                return state["gen"] is None and state["idx"] >= len(unit_specs)

            ob_sb = {}
            ag_deps = {}

            def emit_attn_qk(b, h, qc, pT):
                qoff = h * T + b * L + qc * CH
                koff = h * T + b * L
                nkt = 4 * qc + 4
                for kt in range(nkt):
                    off = max(0, kt * P - qc * CH)
                    sps = psA.tile([P, CH], FP, name="sps", tag="A")
                    nc.tensor.matmul(
                        sps[:, off:], kt_sb[:, koff + kt * P: koff + (kt + 1) * P],
                        qt_sb[:, qoff + off: qoff + CH], start=True, stop=True)
                    nc.scalar.activation(
                        pT[:, kt, off:], sps[:, off:],
                        mybir.ActivationFunctionType.Exp, scale=SCALE)
                    if kt >= 4 * qc:
                        # causal mask: zero pT where k (partition) > q (free)
                        nc.gpsimd.affine_select(
                            out=pT[:, kt, off:off + P], in_=pT[:, kt, off:off + P],
                            compare_op=mybir.AluOpType.is_ge, fill=0.0,
                            base=0, pattern=[[1, P]], channel_multiplier=-1)
                    if off:
                        nc.gpsimd.memset(pT[:, kt, :off], 0.0)
                    fill_mms(2)

            def emit_attn_pv(b, h, qc, pT):
                nkt = 4 * qc + 4
                ops = psO.tile([P, CH], FP, name="ops", tag="O")
                sps2 = psS.tile([P, CH], FP, name="sps2", tag="S")
                for kt in range(nkt):
                    nc.tensor.matmul(
                        ops[:, :], v3[:, b * N_LT + kt, h * HD:(h + 1) * HD],
                        pT[:, kt, :], start=(kt == 0), stop=(kt == nkt - 1))
                    nc.tensor.matmul(
                        sps2[:, :], ones_sb[:, :], pT[:, kt, :],
                        start=(kt == 0), stop=(kt == nkt - 1))
                rec = rec_pool.tile([P, CH], FP, name="rec", tag="rec")
                nc.vector.reciprocal(rec[:, :], sps2[:, :])
                obcp = nc.vector.tensor_tensor(
                    ob_sb[b][:, h * L + qc * CH: h * L + (qc + 1) * CH],
                    ops[:, :], rec[:, :], op=mybir.AluOpType.mult)
                ag_deps[(b, h, qc)] = obcp

            def post_ag(b, c0, c1):
                for h in range(HPC):
                    nc.sync.dma_start(
                        out=o_bounce[(b, c0)][h * HD:(h + 1) * HD, :],
                        in_=ob_sb[b][:, h * L + c0 * CH: h * L + c1 * CH])
                nc.gpsimd.collective_compute(
                    "AllGather", mybir.AluOpType.bypass,
                    ins=[o_bounce[(b, c0)][:]],
                    outs=[ag_o[(b, c0)][:]],
                    replica_groups=rg)

            ph3_pools = {}

            def ph3_unit(b, c0, tch, dep=None):
                if "ot" not in ph3_pools:
                    ph3_pools["ot"] = tc.alloc_tile_pool(name="ot", bufs=2)
                    ph3_pools["fo"] = tc.alloc_tile_pool(name="fo", bufs=2)
                ot = ph3_pools["ot"].tile([P, N_DT, CH], BF, name="ot", tag="ot")
                for dt in range(N_DT):
                    d = nc.sync.dma_start(
                        out=ot[:, dt, :],
                        in_=ag_o[(b, c0)][dt * P:(dt + 1) * P,
                                          tch * CH:(tch + 1) * CH])
                    if dep is not None and dt == 0:
                        add_dep_helper(d.ins, dep.ins,
                                       reason="stagger ph3 behind AG")
                t0 = b * L + (c0 + tch) * CH
                for et in range(HPC):
                    f_ps = psB.tile([P, CH], FP, name="f_ps", tag="B")
                    for dt in range(N_DT):
                        nc.tensor.matmul(
                            f_ps[:, :], wo3[:, dt, et * P:(et + 1) * P],
                            ot[:, dt, :],
                            start=(dt == 0), stop=(dt == N_DT - 1))
                    f_sb = ph3_pools["fo"].tile([P, CH], FP, name="f_sb", tag="f")
                    nc.vector.tensor_copy(f_sb[:, :], f_ps[:, :])
                    nc.sync.dma_start(
                        out=out_d[et * P:(et + 1) * P, t0:t0 + CH],
                        in_=f_sb[:, :])

            # ---------------- emission schedule ----------------
            # x streams on the Sync HWDGE queue; weights/trig on the Scalar
            # HWDGE queue in parallel (ScalarE is idle at start).
            load_x(0, split=True)
            nc.scalar.dma_start(out=w3[:, :, 0:2 * P], in_=w_d3[:, :, 0:2 * P])
            nc.scalar.dma_start(out=trig_sb[("c", 0)][:, :], in_=trig_d[("c", 0)][:, :])
            nc.scalar.dma_start(out=trig_sb[("s", 0)][:, :], in_=trig_d[("s", 0)][:, :])
            nc.scalar.dma_start(out=w3[:, :, 2 * P:4 * P], in_=w_d3[:, :, 2 * P:4 * P])
            nc.scalar.dma_start(out=w3[:, :, 4 * P:6 * P], in_=w_d3[:, :, 4 * P:6 * P])
            nc.scalar.dma_start(out=trig_sb[("c", 1)][:, :], in_=trig_d[("c", 1)][:, :])
            nc.scalar.dma_start(out=trig_sb[("s", 1)][:, :], in_=trig_d[("s", 1)][:, :])
            nc.scalar.dma_start(out=woT_sb[:, :], in_=woT_d[:, :])
            ob_sb[0] = ob_pool.tile([P, HPC * L], BF, name="ob_sb0", tag="ob")
            ob_sb[1] = ob_pool.tile([P, HPC * L], BF, name="ob_sb1", tag="ob")

            # chunk 0 emitted eagerly (not part of the filler stream)
            for g in range(4):
                for _ in ph1_qk_unit(0, g):
                    pass
            for tl in range(4):
                for _ in ph1_v_unit(0, tl):
                    pass
            load_x(1)
            load_x(2)

            ph3_fill = []
            for (c0, c1) in AG_PIECES[0]:
                for tch in range(c1 - c0):
                    ph3_fill.append((0, c0, tch))

            for b in range(B):
                for qc in range(4):
                    drain_ph1_through(b * 4 + qc)
                    if b == 1 and qc >= 2 and not state["released"]:
                        # all phase-1 work is emitted; free its SBUF for ph3
                        drain_ph1_through(N_CH - 1)
                        assert ph1_done()
                        rtmp_pool.release()
                        xin_pool.release()
                        w1_pool.release()
                        state["released"] = True
                    for h in range(HPC):
                        pT = pT_pool.tile([P, N_LT, CH], BF, name="pT", tag="pT")
                        emit_attn_qk(b, h, qc, pT)
                        if b == 0 or not state["released"]:
                            fill_mms(32)
                        elif ph3_fill:
                            b_, c0_, tch_ = ph3_fill.pop(0)
                            ph3_unit(b_, c0_, tch_,
                                     dep=ag_deps.get((1, 0, max(0, qc - 1))))
                        emit_attn_pv(b, h, qc, pT)
                    for (c0, c1) in AG_PIECES[b]:
                        if c1 == qc + 1:
                            post_ag(b, c0, c1)

            for b_, c0_, tch_ in ph3_fill:
                ph3_unit(b_, c0_, tch_)
            for (c0, c1) in AG_PIECES[1]:
                for tch in range(c1 - c0):
                    ph3_unit(1, c0, tch)

            for pool in reversed(list(ph3_pools.values())):
                pool.release()

    if hw:
        split_multi_waits(nc)
    return nc


def make_in_maps(x, cos, sin, Wqkv, Wo):
    bf16 = ml_dtypes.bfloat16
    xf = np.ascontiguousarray(np.asarray(x).reshape(T, D)).astype(np.float32)
    xT = xf.T  # [D, T]
    xb = np.ascontiguousarray(
        xT.reshape(N_DT, P, N_CH, CH).transpose(1, 2, 0, 3).reshape(P, -1)
    ).astype(bf16)

    perm = []
    for h in range(HPC):
        perm.extend(h * HD + 2 * np.arange(64))
        perm.extend(h * HD + 2 * np.arange(64) + 1)
    perm = np.asarray(perm)

    cos = np.asarray(cos)
    sin = np.asarray(sin)
    in_maps = []
    for c in range(N_CORES):
        rows = slice(c * ES, (c + 1) * ES)
        wq = Wqkv[rows][perm]
        wk = Wqkv[D + c * ES: D + (c + 1) * ES][perm]
        wv = Wqkv[2 * D + c * ES: 2 * D + (c + 1) * ES]
        wcat = np.concatenate([wq, wk, wv], axis=0)    # [768, D]
        wb = np.ascontiguousarray(
            wcat.T.reshape(N_DT, P, 3 * ES).transpose(1, 0, 2).reshape(P, -1)
        ).astype(bf16)
        im = {"x_d": xb, "w_d": wb}
        for h in range(HPC):
            gh = c * HPC + h
            ce = cos[:, gh * HD + 2 * np.arange(64)].T          # [64, L]
            ss = sin[:, gh * HD + 2 * np.arange(64)].T
            im[f"ce{h}"] = np.ascontiguousarray(
                np.concatenate([ce, ce], axis=0)).astype(bf16)
            im[f"ss{h}"] = np.ascontiguousarray(
                np.concatenate([-ss, ss], axis=0)).astype(bf16)
        wo = Wo[rows]
        wob = np.ascontiguousarray(
            wo.T.reshape(N_DT, P, ES).transpose(1, 0, 2).reshape(P, -1)
        ).astype(bf16)
        im["woT"] = wob
        in_maps.append(im)
    return in_maps


_cache = {}


def kernel(x, cos, sin, Wqkv, Wo):
    from concourse.bass_utils import run_bass_kernel_spmd
    x = np.asarray(x, dtype=np.float32)
    cos = np.asarray(cos, dtype=np.float32)
    sin = np.asarray(sin, dtype=np.float32)
    Wqkv = np.asarray(Wqkv, dtype=np.float32)
    Wo = np.asarray(Wo, dtype=np.float32)
    if "nc" not in _cache:
        _cache["nc"] = build()
    nc = _cache["nc"]
    in_maps = make_in_maps(x, cos, sin, Wqkv, Wo)
    res = run_bass_kernel_spmd(nc, in_maps, core_ids=list(range(N_CORES)))
    pieces = [res.results[c]["out"].T for c in range(N_CORES)]
    return np.concatenate(pieces, axis=1).reshape(B, L, D)
